# revision 4
# baseline (speedup 1.0000x reference)
"""GCN message-passing + global-mean-pool + MLP heads on 8 trn2 NeuronCores.

Sharding: nodes (and their incident edges, partitioned by destination) are
sharded across the 8 cores; the tiny weight matrices are replicated; the
per-graph pooled sums/counts are all-reduced before the MLP heads.

Device pipeline per core:
  phase 1: h' shard = (x_shard @ W) * dinv_shard  -> AllGather full table
  phase 2: for each 128-dst window: indirect-DMA gather h'[src] rows for the
           window's (dst-sorted) edges, scatter-add via one-hot matmuls into
           PSUM, finalize relu(dinv*(agg + selfloop) + b)
  phase 3: pool sums^T/counts via one-hot matmuls, AllReduce [129,64],
           replicated MLP heads (fc1+relu, actor softmax, critic).
"""

import numpy as np

N = 50000
E = 800000
F = 128
H = 128
H2 = 64
T = 8
G = 64
C = 8              # cores
NS = N // C        # 6250 nodes per core
NW = (NS + 127) // 128   # 49 windows of 128 dst nodes
P = 128

_CACHE = {}


# ---------------------------------------------------------------- tile patch
def _install_tilepatch():
    """walrus in this container rejects instructions with >1 sem wait; hoist
    extra waits onto single-wait nops."""
    import bass_rust
    import concourse.mybir as mybir
    import concourse.tile as tile
    from concourse.vector_clock import ScopedClock

    if getattr(tile.TileContext, "_waitsplit_installed", False):
        return
    counter = [0]

    def _split(insts):
        out = []
        for inst in insts:
            si = inst.sync_info
            waits = list(si.on_wait) if si is not None else []
            if len(waits) > 1:
                for w in waits[:-1]:
                    counter[0] += 1
                    out.append(mybir.InstNoOp(
                        name=f"waitsplit-{counter[0]}", bass_nofuse=True,
                        engine=inst.engine, ins=[], outs=[],
                        sync_info=bass_rust.SyncInfo(on_wait=[w], on_update=[])))
                inst.sync_info = bass_rust.SyncInfo(
                    on_wait=[waits[-1]], on_update=list(si.on_update))
            out.append(inst)
        return out

    orig_lower = tile.TileContext._lower_ordered_insts

    def patched_lower(self, ordered):
        for k in list(ordered.keys()):
            ordered[k] = _split(ordered[k])
        return orig_lower(self, ordered)

    def patched_drain(self, tick_clock, wait_clock):
        nc = self.nc
        probe = nc.sync.nop(nofuse=True)
        wait_clock.add_sem_waits(probe.ins, ScopedClock({None: tick_clock.global_clock}))
        si = probe.ins.sync_info
        waits = list(si.on_wait) if si is not None else []
        if len(waits) > 1:
            probe.ins.sync_info = bass_rust.SyncInfo(
                on_wait=[waits[0]], on_update=list(si.on_update))
            for w in waits[1:]:
                extra = nc.sync.nop(nofuse=True)
                extra.ins.sync_info = bass_rust.SyncInfo(on_wait=[w], on_update=[])
        nc.sync.drain()
        nc.all_engine_barrier()
        popped = nc._tile_sem_poison_stack.pop()
        assert popped is self._sem_poison
        nc.clear_and_free_semaphores(list(self.sems.allocated().values()))
        nc.all_engine_barrier()

    tile.TileContext._lower_ordered_insts = patched_lower
    tile.TileContext._drain_and_barrier = patched_drain
    tile.TileContext._waitsplit_installed = True


# ---------------------------------------------------------------- bass build
def _build(tiles_per_window):
    import concourse.bass as bass
    import concourse.mybir as mybir
    import concourse.tile as tile
    from concourse.masks import make_identity

    _install_tilepatch()
    f32 = mybir.dt.float32
    i32 = mybir.dt.int32
    TT = int(sum(tiles_per_window))

    nc = bass.Bass()
    x_sh = nc.declare_dram_parameter("x_sh", [NS, F], f32, isOutput=False)
    idxs = nc.declare_dram_parameter("idxs", [P, TT], i32, isOutput=False)
    dstrel = nc.declare_dram_parameter("dstrel", [P, TT], f32, isOutput=False)
    batchw = nc.declare_dram_parameter("batchw", [P, NW], f32, isOutput=False)
    dinvw = nc.declare_dram_parameter("dinvw", [P, NW], f32, isOutput=False)
    W_p = nc.declare_dram_parameter("W", [F, H], f32, isOutput=False)
    b_p = nc.declare_dram_parameter("b", [1, H], f32, isOutput=False)
    fc1w_p = nc.declare_dram_parameter("fc1_w", [H, H2], f32, isOutput=False)
    fc1b_p = nc.declare_dram_parameter("fc1_b", [1, H2], f32, isOutput=False)
    aw_p = nc.declare_dram_parameter("actor_w", [H2, T], f32, isOutput=False)
    ab_p = nc.declare_dram_parameter("actor_b", [1, T], f32, isOutput=False)
    cw_p = nc.declare_dram_parameter("critic_w", [H2, 1], f32, isOutput=False)
    cb_p = nc.declare_dram_parameter("critic_b", [1, 1], f32, isOutput=False)
    out_probs = nc.declare_dram_parameter("probs", [G, T], f32, isOutput=True)
    out_value = nc.declare_dram_parameter("value", [G, 1], f32, isOutput=True)

    hshard = nc.dram_tensor("hshard", [NS, H], f32)
    table = nc.dram_tensor("table", [N, H], f32, addr_space="Shared")
    pr = nc.dram_tensor("pr", [H + 1, G], f32)
    pr_red = nc.dram_tensor("pr_red", [H + 1, G], f32, addr_space="Shared")
    groups = [list(range(C))]

    with tile.TileContext(nc) as tc:
        with (
            tc.tile_pool(name="const", bufs=1) as cpool,
            tc.tile_pool(name="hl", bufs=1) as hlpool,
            tc.tile_pool(name="work", bufs=4) as wpool,
            tc.tile_pool(name="gather", bufs=8) as gpool,
            tc.tile_pool(name="psum", bufs=2, space="PSUM") as ppool,
            tc.tile_pool(name="psacc", bufs=1, space="PSUM") as papool,
        ):
            ident = cpool.tile([P, P], f32)
            make_identity(nc, ident[:])
            W_t = cpool.tile([F, H], f32)
            nc.sync.dma_start(out=W_t[:], in_=W_p[:])
            dinv_t = cpool.tile([P, NW], f32)
            nc.sync.dma_start(out=dinv_t[:], in_=dinvw[:])

            # ---------------- phase 1: h' shard + allgather
            hls = []
            for w in range(NW):
                r0 = w * P
                nrow = min(P, NS - r0)
                x_t = wpool.tile([P, F], f32, tag="xt")
                if nrow < P:
                    nc.vector.memset(x_t[:], 0.0)
                nc.sync.dma_start(out=x_t[:nrow, :], in_=x_sh[r0:r0 + nrow, :])
                xT_ps = ppool.tile([P, P], f32, tag="ph1")
                nc.tensor.transpose(out=xT_ps[:], in_=x_t[:], identity=ident[:])
                xT = wpool.tile([P, P], f32, tag="xTs")
                nc.vector.tensor_copy(out=xT[:], in_=xT_ps[:])
                h_ps = ppool.tile([P, H], f32, tag="ph1")
                nc.tensor.matmul(out=h_ps[:], lhsT=xT[:], rhs=W_t[:],
                                 start=True, stop=True)
                hl = hlpool.tile([P, H], f32, tag=f"hl{w}")
                nc.vector.tensor_tensor(
                    out=hl[:], in0=h_ps[:],
                    in1=dinv_t[:, w:w + 1].to_broadcast([P, H]),
                    op=mybir.AluOpType.mult)
                nc.sync.dma_start(out=hshard[r0:r0 + nrow, :], in_=hl[:nrow, :])
                hls.append(hl)

            nc.gpsimd.collective_compute(
                "AllGather", mybir.AluOpType.bypass, replica_groups=groups,
                ins=[hshard[:]], outs=[table[:]])

            # ---------------- phase 2 consts
            iota128 = cpool.tile([P, P], f32)
            nc.gpsimd.iota(out=iota128[:], pattern=[[1, P]], base=0,
                           channel_multiplier=0,
                           allow_small_or_imprecise_dtypes=True)
            iota64 = cpool.tile([P, G], f32)
            nc.gpsimd.iota(out=iota64[:], pattern=[[1, G]], base=0,
                           channel_multiplier=0,
                           allow_small_or_imprecise_dtypes=True)
            ones_r = cpool.tile([1, P], f32)
            nc.vector.memset(ones_r[:], 1.0)
            ones_c = cpool.tile([P, 1], f32)
            nc.vector.memset(ones_c[:], 1.0)
            b_t = cpool.tile([1, H], f32)
            nc.sync.dma_start(out=b_t[:], in_=b_p[:])
            bb_ps = ppool.tile([P, H], f32, tag="head")
            nc.tensor.matmul(out=bb_ps[:], lhsT=ones_r[:], rhs=b_t[:],
                             start=True, stop=True)
            B_bc = cpool.tile([P, H], f32)
            nc.vector.tensor_copy(out=B_bc[:], in_=bb_ps[:])
            idx_t = cpool.tile([P, TT], i32)
            nc.sync.dma_start(out=idx_t[:], in_=idxs[:])
            dr_t = cpool.tile([P, TT], f32)
            nc.sync.dma_start(out=dr_t[:], in_=dstrel[:])
            bat_t = cpool.tile([P, NW], f32)
            nc.sync.dma_start(out=bat_t[:], in_=batchw[:])

            pool_ps = papool.tile([H, G], f32, tag="pool")
            cnt_ps = papool.tile([1, G], f32, tag="cnt")

            # ---------------- phase 2: gather/scatter per window
            gt = 0
            for w in range(NW):
                tw = int(tiles_per_window[w])
                agg_ps = ppool.tile([P, H], f32, tag="agg")
                for t in range(tw):
                    g = gpool.tile([P, H], f32, tag="g")
                    nc.gpsimd.indirect_dma_start(
                        out=g[:], out_offset=None, in_=table[:],
                        in_offset=bass.IndirectOffsetOnAxis(
                            ap=idx_t[:, gt:gt + 1], axis=0))
                    S = gpool.tile([P, P], f32, tag="S")
                    nc.vector.tensor_tensor(
                        out=S[:], in0=dr_t[:, gt:gt + 1].to_broadcast([P, P]),
                        in1=iota128[:], op=mybir.AluOpType.is_equal)
                    nc.tensor.matmul(out=agg_ps[:], lhsT=S[:], rhs=g[:],
                                     start=(t == 0), stop=(t == tw - 1))
                    gt += 1
                t1 = wpool.tile([P, H], f32, tag="t1")
                nc.vector.tensor_tensor(out=t1[:], in0=agg_ps[:], in1=hls[w][:],
                                        op=mybir.AluOpType.add)
                t2 = wpool.tile([P, H], f32, tag="t2")
                nc.vector.tensor_tensor(
                    out=t2[:], in0=t1[:],
                    in1=dinv_t[:, w:w + 1].to_broadcast([P, H]),
                    op=mybir.AluOpType.mult)
                t3 = wpool.tile([P, H], f32, tag="t3")
                nc.vector.tensor_tensor(out=t3[:], in0=t2[:], in1=B_bc[:],
                                        op=mybir.AluOpType.add)
                ho = wpool.tile([P, H], f32, tag="ho")
                nc.vector.tensor_scalar(out=ho[:], in0=t3[:], scalar1=0.0,
                                        scalar2=None, op0=mybir.AluOpType.max)
                Mw = wpool.tile([P, G], f32, tag="Mw")
                nc.vector.tensor_tensor(
                    out=Mw[:], in0=bat_t[:, w:w + 1].to_broadcast([P, G]),
                    in1=iota64[:], op=mybir.AluOpType.is_equal)
                nc.tensor.matmul(out=pool_ps[:], lhsT=ho[:], rhs=Mw[:],
                                 start=(w == 0), stop=(w == NW - 1))
                nc.tensor.matmul(out=cnt_ps[:], lhsT=ones_c[:], rhs=Mw[:],
                                 start=(w == 0), stop=(w == NW - 1))

            # ---------------- phase 3: allreduce + heads
            pool_sb = wpool.tile([H, G], f32, tag="pools")
            nc.vector.tensor_copy(out=pool_sb[:], in_=pool_ps[:])
            cnt_sb = wpool.tile([1, G], f32, tag="cnts")
            nc.vector.tensor_copy(out=cnt_sb[:], in_=cnt_ps[:])
            nc.sync.dma_start(out=pr[0:H, :], in_=pool_sb[:])
            nc.sync.dma_start(out=pr[H:H + 1, :], in_=cnt_sb[:])
            nc.gpsimd.collective_compute(
                "AllReduce", mybir.AluOpType.add, replica_groups=groups,
                ins=[pr[:]], outs=[pr_red[:]])

            sums_t = wpool.tile([H, G], f32, tag="sumr")
            nc.sync.dma_start(out=sums_t[:], in_=pr_red[0:H, :])
            cntr = wpool.tile([1, G], f32, tag="cntr")
            nc.sync.dma_start(out=cntr[:], in_=pr_red[H:H + 1, :])

            cmax = wpool.tile([1, G], f32, tag="cmax")
            nc.vector.tensor_scalar(out=cmax[:], in0=cntr[:], scalar1=1.0,
                                    scalar2=None, op0=mybir.AluOpType.max)
            recip = wpool.tile([1, G], f32, tag="recip")
            nc.vector.reciprocal(out=recip[:], in_=cmax[:])
            rb_ps = ppool.tile([P, G], f32, tag="head")
            nc.tensor.matmul(out=rb_ps[:], lhsT=ones_r[:], rhs=recip[:],
                             start=True, stop=True)
            rb_sb = wpool.tile([P, G], f32, tag="rbs")
            nc.vector.tensor_copy(out=rb_sb[:], in_=rb_ps[:])
            gsT = wpool.tile([H, G], f32, tag="gsT")
            nc.vector.tensor_tensor(out=gsT[:], in0=sums_t[:], in1=rb_sb[:],
                                    op=mybir.AluOpType.mult)

            fc1w_t = cpool.tile([H, H2], f32)
            nc.sync.dma_start(out=fc1w_t[:], in_=fc1w_p[:])
            fc1b_t = cpool.tile([1, H2], f32)
            nc.sync.dma_start(out=fc1b_t[:], in_=fc1b_p[:])
            ones_g = cpool.tile([1, G], f32)
            nc.vector.memset(ones_g[:], 1.0)
            z_ps = ppool.tile([H2, G], f32, tag="head")
            nc.tensor.matmul(out=z_ps[:], lhsT=fc1w_t[:], rhs=gsT[:],
                             start=True, stop=False)
            nc.tensor.matmul(out=z_ps[:], lhsT=fc1b_t[:], rhs=ones_g[:],
                             start=False, stop=True)
            zt = wpool.tile([H2, G], f32, tag="zt")
            nc.vector.tensor_scalar(out=zt[:], in0=z_ps[:], scalar1=0.0,
                                    scalar2=None, op0=mybir.AluOpType.max)

            aw_t = cpool.tile([H2, T], f32)
            nc.sync.dma_start(out=aw_t[:], in_=aw_p[:])
            ab_t = cpool.tile([1, T], f32)
            nc.sync.dma_start(out=ab_t[:], in_=ab_p[:])
            ones_g2 = cpool.tile([1, H2], f32)
            nc.vector.memset(ones_g2[:], 1.0)
            lg_ps = ppool.tile([G, T], f32, tag="head")
            nc.tensor.matmul(out=lg_ps[:], lhsT=zt[:], rhs=aw_t[:],
                             start=True, stop=False)
            nc.tensor.matmul(out=lg_ps[:], lhsT=ones_g2[:], rhs=ab_t[:],
                             start=False, stop=True)
            rmax = wpool.tile([G, 1], f32, tag="rmax")
            nc.vector.tensor_reduce(out=rmax[:], in_=lg_ps[:],
                                    axis=mybir.AxisListType.X,
                                    op=mybir.AluOpType.max)
            sh = wpool.tile([G, T], f32, tag="sh")
            nc.vector.tensor_tensor(out=sh[:], in0=lg_ps[:],
                                    in1=rmax[:].to_broadcast([G, T]),
                                    op=mybir.AluOpType.subtract)
            ex = wpool.tile([G, T], f32, tag="ex")
            nc.scalar.activation(out=ex[:], in_=sh[:],
                                 func=mybir.ActivationFunctionType.Exp)
            rsum = wpool.tile([G, 1], f32, tag="rsum")
            nc.vector.tensor_reduce(out=rsum[:], in_=ex[:],
                                    axis=mybir.AxisListType.X,
                                    op=mybir.AluOpType.add)
            rrec = wpool.tile([G, 1], f32, tag="rrec")
            nc.vector.reciprocal(out=rrec[:], in_=rsum[:])
            probs = wpool.tile([G, T], f32, tag="probs")
            nc.vector.tensor_tensor(out=probs[:], in0=ex[:],
                                    in1=rrec[:].to_broadcast([G, T]),
                                    op=mybir.AluOpType.mult)
            nc.sync.dma_start(out=out_probs[:], in_=probs[:])

            cw_t = cpool.tile([H2, 1], f32)
            nc.sync.dma_start(out=cw_t[:], in_=cw_p[:])
            cb_t = cpool.tile([1, 1], f32)
            nc.sync.dma_start(out=cb_t[:], in_=cb_p[:])
            val_ps = ppool.tile([G, 1], f32, tag="head")
            nc.tensor.matmul(out=val_ps[:], lhsT=zt[:], rhs=cw_t[:],
                             start=True, stop=False)
            nc.tensor.matmul(out=val_ps[:], lhsT=ones_g2[:], rhs=cb_t[:],
                             start=False, stop=True)
            val_sb = wpool.tile([G, 1], f32, tag="vals")
            nc.vector.tensor_copy(out=val_sb[:], in_=val_ps[:])
            nc.sync.dma_start(out=out_value[:], in_=val_sb[:])

    return nc


# ---------------------------------------------------------------- host side
def kernel(x, edge_index, batch, W, b, fc1_w, fc1_b, actor_w, actor_b,
           critic_w, critic_b):
    from concourse.bass_utils import run_bass_kernel_spmd

    x = np.asarray(x, np.float32)
    edge_index = np.asarray(edge_index, np.int32)
    batch = np.asarray(batch, np.int32)

    src, dst = edge_index[0].astype(np.int64), edge_index[1].astype(np.int64)
    deg = (np.bincount(dst, minlength=N) + 1).astype(np.float32)
    dinv = deg ** -0.5

    # per-core edge-cut partitioning, dst-sorted, window-aligned padding
    core_of = dst // NS
    order = np.argsort(dst, kind="stable")
    src_s, dst_s = src[order], dst[order]
    core_s = core_of[order]
    counts = np.zeros((C, NW), np.int64)
    per_core = []
    for c in range(C):
        m = core_s == c
        sc, dc = src_s[m], dst_s[m] - c * NS
        wc = dc // P
        counts[c] = np.bincount(wc, minlength=NW)
        per_core.append((sc, dc, wc))
    tiles_per_window = np.maximum(1, (counts.max(axis=0) + P - 1) // P)
    TT = int(tiles_per_window.sum())

    idx_np = np.zeros((C, P, TT), np.int32)
    dr_np = np.full((C, P, TT), -1.0, np.float32)
    wbase = np.concatenate([[0], np.cumsum(tiles_per_window)]).astype(np.int64)
    for c in range(C):
        sc, dc, wc = per_core[c]
        # position within window stream
        pos_in_w = np.arange(len(sc)) - np.concatenate(
            [[0], np.cumsum(counts[c])])[wc]
        tile_i = wbase[wc] + pos_in_w // P
        part_i = pos_in_w % P
        idx_np[c, part_i, tile_i] = sc
        dr_np[c, part_i, tile_i] = (dc - wc * P).astype(np.float32)

    bat_np = np.full((C, P, NW), -1.0, np.float32)
    dinv_np = np.zeros((C, P, NW), np.float32)
    for c in range(C):
        loc = np.arange(NS)
        bat_np[c, loc % P, loc // P] = batch[c * NS:(c + 1) * NS].astype(np.float32)
        dinv_np[c, loc % P, loc // P] = dinv[c * NS:(c + 1) * NS]

    key = tuple(int(t) for t in tiles_per_window)
    if key not in _CACHE:
        _CACHE.clear()
        _CACHE[key] = _build(tiles_per_window)
    nc = _CACHE[key]

    base = {
        "idxs": None, "dstrel": None, "x_sh": None, "batchw": None,
        "dinvw": None,
        "W": np.asarray(W, np.float32),
        "b": np.asarray(b, np.float32).reshape(1, H),
        "fc1_w": np.asarray(fc1_w, np.float32),
        "fc1_b": np.asarray(fc1_b, np.float32).reshape(1, H2),
        "actor_w": np.asarray(actor_w, np.float32),
        "actor_b": np.asarray(actor_b, np.float32).reshape(1, T),
        "critic_w": np.asarray(critic_w, np.float32),
        "critic_b": np.asarray(critic_b, np.float32).reshape(1, 1),
    }
    in_maps = []
    for c in range(C):
        m = dict(base)
        m["x_sh"] = np.ascontiguousarray(x[c * NS:(c + 1) * NS])
        m["idxs"] = np.ascontiguousarray(idx_np[c])
        m["dstrel"] = np.ascontiguousarray(dr_np[c])
        m["batchw"] = np.ascontiguousarray(bat_np[c])
        m["dinvw"] = np.ascontiguousarray(dinv_np[c])
        in_maps.append(m)

    res = run_bass_kernel_spmd(nc, in_maps, core_ids=list(range(C)))
    r0 = res.results[0]
    return np.asarray(r0["probs"]), np.asarray(r0["value"])
